# revision 24
# baseline (speedup 1.0000x reference)
"""Bahdanau-style attention scores kernel for 8 TRN2 NeuronCores.

Reference math (B=64, S=2048, E=512, D=512):
    Wh = attn_W[:D]; We = attn_W[D:]
    h_proj = hidden @ Wh                                  # [B, D]
    e_proj[b,s,:] = enc[b,s,:] @ We                       # [B, S, D]
    energy = tanh(h_proj[:,None,:] + e_proj + attn_b)     # [B, S, D]
    scores = energy @ v_w                                 # [B, S]
    out = softmax(scores, axis=1)

Sharding: data-parallel over batch, 8 batches per core.
Host precomputes c = hidden @ Wh + attn_b (tiny: 33 MFLOP), converts
enc/We to bf16 (halves HBM traffic; absmax_rel ~5e-3, gate is 2e-2),
lays enc out partition-major per s-block ([SC, P, EC, ST], giving the
DMA 4 KB contiguous DRAM runs instead of 1 KB -> full ~358 GB/s at
startup) and We as one p-major [P, EC*D] block, and applies the final
softmax to the raw scores the device returns (scores are 0.2% of the
FLOPs; this removes the psum zero opener/closer matmuls and the
exp/reduce/reciprocal tail chain from the device critical path).

Per-core kernel (b = 0..7 local batches):
  b0 runs s-outer / e-grouped / d-inner so matmuls only need the
  s-blocks that have already landed; warm-up matmuls (dummy data)
  bridge the ~5 us DMA latency+transfer window for we+s0 and keep the
  PE continuously busy so the HAM clock-gate releases (1.2 -> 2.4 GHz)
  by the time real work starts. b1..b7 run d-outer / e-outer (4
  consecutive matmuls share lhsT).
  PSUM: 6 single-bank [128, 512] tiles (+2 score banks); each (d, s)
  accumulation gets its own bank and tanh ACT call ([128,512] f32 ->
  bf16 energy, bias c[b,d]) for smooth bank rotation.
  Score matvecs (v_d.T @ energyT, 4 s-strips concurrent via
  tile_position column groups) are deferred one d-block and flushed
  1-2 rounds at MM #14 of each d-block, where they pipeline at the
  ~215 ns full-width issue rate. After a batch's last round, DVE
  copies the 4 score rows psum->sbuf and a small DMA writes them out.
  The last batch's scores live in two psum banks (strips s0,s1 vs
  s2,s3) so the tail's row copies don't serialize against later strip
  matvecs (Tile's WAR tracking is tile-granular), its last d-block is
  s-grouped so tanh overlaps the matmuls, and the final copies
  alternate DVE/ACT: the exposed tail is ~2 us.
"""

import numpy as np

import concourse.bass as bass  # noqa: F401  (engine namespaces via nc)
import concourse.mybir as mybir
import concourse.tile as tile
from concourse import bacc
from concourse.bass_utils import run_bass_kernel_spmd

N_CORES = 8
B, S, E, D = 64, 2048, 512, 512
BL = B // N_CORES          # local batches per core
P = 128                    # partition tile
EC = E // P                # e chunks (4)
DC = D // P                # d chunks (4)
ST = 512                   # s tile (free dim per matmul; one PSUM bank f32)
SC = S // ST               # s tiles (4)

DT = mybir.dt.bfloat16     # matmul input dtype (enc, We, v, energy)

_COMPILED = None  # nc cache within the process


def _build(warmup=11, enc_bufs=3, psp_bufs=6, ens_bufs=24):
    nc = bacc.Bacc(
        "TRN2", target_bir_lowering=False, debug=False, num_devices=N_CORES
    )
    f32 = mybir.dt.float32

    enc_ap = nc.dram_tensor(
        "enc_t", [BL, SC, P, EC, ST], DT, kind="ExternalInput"
    ).ap()
    we_ap = nc.dram_tensor("we", [P, EC * D], DT, kind="ExternalInput").ap()
    c_ap = nc.dram_tensor("c", [P, BL * DC], f32, kind="ExternalInput").ap()
    v_ap = nc.dram_tensor("v", [P, DC], DT, kind="ExternalInput").ap()
    out_ap = nc.dram_tensor("out", [BL, S], f32, kind="ExternalOutput").ap()

    with tile.TileContext(nc) as tc:
        with (
            tc.tile_pool(name="singles", bufs=1) as singles,
            tc.tile_pool(name="encp", bufs=enc_bufs) as encp,
            tc.tile_pool(name="ensp", bufs=ens_bufs) as ensp,
            tc.tile_pool(name="outp", bufs=2) as outp,
            tc.tile_pool(name="warmp", bufs=1) as warmp,
            tc.tile_pool(name="psp", bufs=psp_bufs, space="PSUM") as psp,
            tc.tile_pool(name="scp", bufs=2, space="PSUM") as scp,
        ):
            # ---- PE warm-up: keep the PE busy while we + the first enc
            # s-block land (dummy data, never read). gpsimd does the memset
            # (its preamble retires earliest).
            wsb = warmp.tile([P, ST], DT)
            nc.gpsimd.memset(wsb, 0.0)
            wps = psp.tile([P, ST], f32, name="warm_ps", tag="ps")
            for _ in range(warmup):
                nc.tensor.matmul(wps, lhsT=wsb[:, 0:P], rhs=wsb,
                                 start=True, stop=True)

            we_sb = singles.tile([P, EC, D], DT)
            c_sb = singles.tile([P, BL * DC], f32)
            v_sb = singles.tile([P, DC], DT)

            rounds = []  # FIFO of (tiles-or-tile, d, [en(d,s) for s], batch)

            def emit_out(scores_t, b):
                # stage the 4 score rows to SBUF (DVE) and DMA them out;
                # host does the softmax.
                outw = outp.tile([P, ST], f32, name=f"outw_b{b}", tag="outw")
                for s in range(SC):
                    nc.vector.tensor_scalar_mul(
                        outw[32 * s : 32 * s + 1, :],
                        scores_t[32 * s : 32 * s + 1, :],
                        1.0,
                    )
                nc.sync.dma_start(
                    out=out_ap[b].rearrange("(r s) -> r s", r=SC),
                    in_=outw[0 : 32 * (SC - 1) + 1 : 32, :],
                )

            def flush_one():
                scores_t, d, ens, b = rounds.pop(0)
                tiles = scores_t if isinstance(scores_t, list) else (
                    [scores_t] * SC)
                for s in range(SC):
                    nc.tensor.matmul(
                        tiles[s][32 * s : 32 * s + 1, :],
                        lhsT=v_sb[:, d : d + 1],
                        rhs=ens[s],
                        start=(d == 0),
                        stop=(d == DC - 1),
                        tile_position=(0, 32 * s),
                    )
                if d == DC - 1 and not isinstance(scores_t, list):
                    emit_out(scores_t, b)

            def slot(k=2):
                for _ in range(min(k, len(rounds))):
                    flush_one()

            for b in range(BL):
                enc_sb = encp.tile([P, EC, SC, ST], DT, name=f"enc_b{b}",
                                   tag="enc")
                if b == 0:
                    # split we and s0 so the first matmuls' deps land early
                    we_flat = we_sb.rearrange("p e d -> p (e d)")
                    nc.sync.dma_start(out=we_flat[:, 0 : 2 * D],
                                      in_=we_ap[:, 0 : 2 * D])
                    nc.sync.dma_start(out=enc_sb[:, 0:2, 0, :],
                                      in_=enc_ap[b, 0][:, 0:2, :])
                    nc.sync.dma_start(out=we_flat[:, 2 * D :],
                                      in_=we_ap[:, 2 * D :])
                    nc.sync.dma_start(out=enc_sb[:, 2:4, 0, :],
                                      in_=enc_ap[b, 0][:, 2:4, :])
                    nc.sync.dma_start(out=c_sb, in_=c_ap)
                    nc.sync.dma_start(out=v_sb, in_=v_ap)
                    rest = range(1, SC)
                else:
                    rest = range(SC)
                for sb in rest:
                    nc.sync.dma_start(
                        out=enc_sb[:, :, sb, :],
                        in_=enc_ap[b, sb],
                    )

                if b == BL - 1:
                    sc_lo = scp.tile([P, ST], f32, name="scores_lo", tag="sc")
                    sc_hi = scp.tile([P, ST], f32, name="scores_hi", tag="sc")
                    scores = [sc_lo, sc_lo, sc_hi, sc_hi]
                else:
                    scores = scp.tile([P, ST], f32, name=f"scores_b{b}",
                                      tag="sc")

                def tanh_en(ps_t, d, s):
                    en = ensp.tile([P, ST], DT, name=f"en_b{b}d{d}s{s}",
                                   tag="ens")
                    nc.scalar.activation(
                        en,
                        ps_t,
                        mybir.ActivationFunctionType.Tanh,
                        bias=c_sb[:, b * DC + d : b * DC + d + 1],
                    )
                    return en

                if b == 0:
                    # s-outer / e-grouped / d-inner: matmuls for s-block s
                    # need only we + enc s-block s
                    en_by_d = [[None] * SC for _ in range(DC)]
                    for s in range(SC):
                        ps4 = [
                            psp.tile([P, ST], f32, name=f"ps_b0s{s}d{d}",
                                     tag="ps")
                            for d in range(DC)
                        ]
                        for e in range(EC):
                            for d in range(DC):
                                nc.tensor.matmul(
                                    ps4[d],
                                    lhsT=we_sb[:, e, d * P : (d + 1) * P],
                                    rhs=enc_sb[:, e, s, :],
                                    start=(e == 0),
                                    stop=(e == EC - 1),
                                )
                                if e == EC - 1:
                                    en_by_d[d][s] = tanh_en(ps4[d], d, s)
                    for d in range(DC):
                        rounds.append((scores, d, en_by_d[d], b))
                else:
                    for d in range(DC):
                        ps4 = [
                            psp.tile([P, ST], f32, name=f"ps_b{b}d{d}s{s}",
                                     tag="ps")
                            for s in range(SC)
                        ]
                        ens = [None] * SC
                        # s-grouped: each s-block's accumulation stops at MM
                        # 4/8/12/16, spreading tanh issue evenly across the
                        # block (smooth ACT pacing, early psum recycle); the
                        # per-MM LDWEIGHTS changes stay hidden at the 215 ns
                        # issue rate
                        order = [(e, s) for s in range(SC)
                                 for e in range(EC)]
                        for i, (e, s) in enumerate(order):
                            nc.tensor.matmul(
                                ps4[s],
                                lhsT=we_sb[:, e, d * P : (d + 1) * P],
                                rhs=enc_sb[:, e, s, :],
                                start=(e == 0),
                                stop=(e == EC - 1),
                            )
                            if e == EC - 1:
                                ens[s] = tanh_en(ps4[s], d, s)
                            if i == 13 and (d == 0 or b == BL - 1):
                                # one 4-round block per batch (fewer strip<->
                                # full-width turnarounds); the last batch
                                # drains per-d so its tail stays short
                                slot(4)
                        rounds.append((scores, d, ens, b))

            # final round (last batch, d3): all 4 strip matvecs, then row
            # copies alternating DVE/ACT. The lo tile's copies only wait on
            # its own strips, so they overlap the hi strips' matvecs.
            tiles, d, ens, b = rounds.pop(0)
            assert d == DC - 1 and not rounds and isinstance(tiles, list)
            for s in range(SC):
                nc.tensor.matmul(
                    tiles[s][32 * s : 32 * s + 1, :],
                    lhsT=v_sb[:, d : d + 1],
                    rhs=ens[s],
                    start=False,
                    stop=True,
                    tile_position=(0, 32 * s),
                )
            outw = outp.tile([P, ST], f32, name="outw_last", tag="outw")
            for s in range(SC):
                src = tiles[s][32 * s : 32 * s + 1, :]
                dst = outw[32 * s : 32 * s + 1, :]
                if s % 2 == 0:
                    nc.vector.tensor_scalar_mul(dst, src, 1.0)
                else:
                    nc.scalar.copy(dst, src)
            nc.sync.dma_start(
                out=out_ap[b].rearrange("(r s) -> r s", r=SC),
                in_=outw[0 : 32 * (SC - 1) + 1 : 32, :],
            )

    nc.compile()
    return nc


def _get_nc():
    global _COMPILED
    if _COMPILED is None:
        _COMPILED = _build()
    return _COMPILED


def _prep_in_maps(hidden, encoder_outputs, attn_W, attn_b, v_w):
    import ml_dtypes

    hidden = np.asarray(hidden, dtype=np.float32)
    encoder_outputs = np.asarray(encoder_outputs, dtype=np.float32)
    attn_W = np.asarray(attn_W, dtype=np.float32)
    attn_b = np.asarray(attn_b, dtype=np.float32)
    v_w = np.asarray(v_w, dtype=np.float32)

    c_full = hidden @ attn_W[:D] + attn_b            # [B, D]
    # We p-major: we[p, e*D + d] = We[e*P + p, d] -> 4 KB DRAM runs
    we = np.ascontiguousarray(
        attn_W[D:].reshape(EC, P, D).transpose(1, 0, 2).reshape(P, EC * D)
    ).astype(ml_dtypes.bfloat16)
    v = np.ascontiguousarray(v_w.reshape(DC, P).T).astype(ml_dtypes.bfloat16)

    in_maps = []
    for i in range(N_CORES):
        lo = i * BL
        enc_bes = encoder_outputs[:, lo : lo + BL, :].transpose(1, 2, 0)
        # [BL, E, S] -> [BL, SC, P, EC, ST]: partition-major per s-block so
        # each partition's DMA run is EC*ST*2 = 4 KB of contiguous DRAM
        enc_t = np.ascontiguousarray(
            enc_bes.reshape(BL, EC, P, SC, ST).transpose(0, 3, 2, 1, 4)
        ).astype(ml_dtypes.bfloat16)
        c_shard = c_full[lo : lo + BL]               # [BL, D]
        c = np.ascontiguousarray(
            c_shard.reshape(BL, DC, P).transpose(2, 0, 1).reshape(P, BL * DC)
        )                                            # [P, BL*DC]
        in_maps.append({"enc_t": enc_t, "we": we, "c": c, "v": v})
    return in_maps


def run_full(inputs: dict, trace: bool = False):
    """Run on 8 cores; returns (full_output [B,S], BassKernelResults)."""
    nc = _get_nc()
    in_maps = _prep_in_maps(**inputs)
    res = run_bass_kernel_spmd(
        nc, in_maps, list(range(N_CORES)), trace=trace
    )
    scores = np.concatenate(
        [res.results[i]["out"] for i in range(N_CORES)], axis=0
    )                                                # [B, S] raw scores
    scores = scores - scores.max(axis=1, keepdims=True)
    ex = np.exp(scores)
    out = (ex / ex.sum(axis=1, keepdims=True)).astype(np.float32)
    return out, res


def kernel(**inputs) -> np.ndarray:
    out, _ = run_full(inputs)
    return out


# revision 26
# speedup vs baseline: 1.1918x; 1.1918x over previous
"""Bahdanau-style attention scores kernel for 8 TRN2 NeuronCores.

Reference math (B=64, S=2048, E=512, D=512):
    Wh = attn_W[:D]; We = attn_W[D:]
    h_proj = hidden @ Wh                                  # [B, D]
    e_proj[b,s,:] = enc[b,s,:] @ We                       # [B, S, D]
    energy = tanh(h_proj[:,None,:] + e_proj + attn_b)     # [B, S, D]
    scores = energy @ v_w                                 # [B, S]
    out = softmax(scores, axis=1)

Sharding: data-parallel over batch, 8 batches per core.
Host precomputes c = hidden @ Wh + attn_b (tiny: 33 MFLOP), converts
enc/We to bf16 (halves HBM traffic; absmax_rel ~5e-3, gate is 2e-2),
lays enc out partition-major per s-block ([SC, P, EC, ST], giving the
DMA 4 KB contiguous DRAM runs instead of 1 KB -> full ~358 GB/s at
startup) and We as one p-major [P, EC*D] block, and applies the final
softmax to the raw scores the device returns (scores are 0.2% of the
FLOPs; this removes the psum zero opener/closer matmuls and the
exp/reduce/reciprocal tail chain from the device critical path).

Per-core kernel (b = 0..7 local batches):
  b0 runs s-outer / e-grouped / d-inner so matmuls only need the
  s-blocks that have already landed; warm-up matmuls (dummy data)
  bridge the ~5 us DMA latency+transfer window for we+s0 and keep the
  PE continuously busy so the HAM clock-gate releases (1.2 -> 2.4 GHz)
  by the time real work starts. b1..b7 run d-outer / e-outer (4
  consecutive matmuls share lhsT).
  PSUM: 6 single-bank [128, 512] tiles (+2 score banks); each (d, s)
  accumulation gets its own bank and tanh ACT call ([128,512] f32 ->
  bf16 energy, bias c[b,d]) for smooth bank rotation.
  Score matvecs (v_d.T @ energyT, 4 s-strips concurrent via
  tile_position column groups) are deferred one batch and flushed as
  one 4-round block at MM #14 of the next batch's d0, where rounds
  pipeline at the ~215 ns full-width issue rate (one block per batch
  minimizes strip<->full-width array turnarounds). After a batch's
  last round, DVE copies the 4 score rows psum->sbuf and a small DMA
  writes them out; the last batch instead drains per-d so its tail
  stays short.
  The last batch's scores live in two psum banks (strips s0,s1 vs
  s2,s3) so the tail's row copies don't serialize against later strip
  matvecs (Tile's WAR tracking is tile-granular), its last d-block is
  s-grouped so tanh overlaps the matmuls, and the final copies
  alternate DVE/ACT: the exposed tail is ~2 us.
"""

import numpy as np

import concourse.bass as bass  # noqa: F401  (engine namespaces via nc)
import concourse.mybir as mybir
import concourse.tile as tile
from concourse import bacc
from concourse.bass_utils import run_bass_kernel_spmd

N_CORES = 8
B, S, E, D = 64, 2048, 512, 512
BL = B // N_CORES          # local batches per core
P = 128                    # partition tile
EC = E // P                # e chunks (4)
DC = D // P                # d chunks (4)
ST = 512                   # s tile (free dim per matmul; one PSUM bank f32)
SC = S // ST               # s tiles (4)

DT = mybir.dt.bfloat16     # matmul input dtype (enc, We, v, energy)

_COMPILED = None  # nc cache within the process


def _build(warmup=11, enc_bufs=3, psp_bufs=6, ens_bufs=24):
    nc = bacc.Bacc(
        "TRN2", target_bir_lowering=False, debug=False, num_devices=N_CORES
    )
    f32 = mybir.dt.float32

    enc_ap = nc.dram_tensor(
        "enc_t", [BL, SC, P, EC, ST], DT, kind="ExternalInput"
    ).ap()
    we_ap = nc.dram_tensor("we", [P, EC * D], DT, kind="ExternalInput").ap()
    c_ap = nc.dram_tensor("c", [P, BL * DC], f32, kind="ExternalInput").ap()
    v_ap = nc.dram_tensor("v", [P, DC], DT, kind="ExternalInput").ap()
    out_ap = nc.dram_tensor("out", [BL, S], f32, kind="ExternalOutput").ap()

    with tile.TileContext(nc) as tc:
        with (
            tc.tile_pool(name="singles", bufs=1) as singles,
            tc.tile_pool(name="encp", bufs=enc_bufs) as encp,
            tc.tile_pool(name="ensp", bufs=ens_bufs) as ensp,
            tc.tile_pool(name="outp", bufs=2) as outp,
            tc.tile_pool(name="warmp", bufs=1) as warmp,
            tc.tile_pool(name="psp", bufs=psp_bufs, space="PSUM") as psp,
            tc.tile_pool(name="scp", bufs=2, space="PSUM") as scp,
        ):
            # ---- PE warm-up: keep the PE busy while we + the first enc
            # s-block land (dummy data, never read). gpsimd does the memset
            # (its preamble retires earliest).
            wsb = warmp.tile([P, ST], DT)
            nc.gpsimd.memset(wsb, 0.0)
            wps = psp.tile([P, ST], f32, name="warm_ps", tag="ps")
            for _ in range(warmup):
                nc.tensor.matmul(wps, lhsT=wsb[:, 0:P], rhs=wsb,
                                 start=True, stop=True)

            we_sb = singles.tile([P, EC, D], DT)
            c_sb = singles.tile([P, BL * DC], f32)
            v_sb = singles.tile([P, DC], DT)

            rounds = []  # FIFO of (tiles-or-tile, d, [en(d,s) for s], batch)

            def emit_out(scores_t, b):
                # stage the 4 score rows to SBUF (DVE) and DMA them out;
                # host does the softmax.
                outw = outp.tile([P, ST], f32, name=f"outw_b{b}", tag="outw")
                for s in range(SC):
                    nc.vector.tensor_scalar_mul(
                        outw[32 * s : 32 * s + 1, :],
                        scores_t[32 * s : 32 * s + 1, :],
                        1.0,
                    )
                nc.sync.dma_start(
                    out=out_ap[b].rearrange("(r s) -> r s", r=SC),
                    in_=outw[0 : 32 * (SC - 1) + 1 : 32, :],
                )

            def flush_one():
                scores_t, d, ens, b = rounds.pop(0)
                tiles = scores_t if isinstance(scores_t, list) else (
                    [scores_t] * SC)
                for s in range(SC):
                    nc.tensor.matmul(
                        tiles[s][32 * s : 32 * s + 1, :],
                        lhsT=v_sb[:, d : d + 1],
                        rhs=ens[s],
                        start=(d == 0),
                        stop=(d == DC - 1),
                        tile_position=(0, 32 * s),
                    )
                if d == DC - 1 and not isinstance(scores_t, list):
                    emit_out(scores_t, b)

            def slot(k=2):
                for _ in range(min(k, len(rounds))):
                    flush_one()

            for b in range(BL):
                enc_sb = encp.tile([P, EC, SC, ST], DT, name=f"enc_b{b}",
                                   tag="enc")
                if b == 0:
                    # split we and s0 so the first matmuls' deps land early
                    we_flat = we_sb.rearrange("p e d -> p (e d)")
                    nc.sync.dma_start(out=we_flat[:, 0 : 2 * D],
                                      in_=we_ap[:, 0 : 2 * D])
                    nc.sync.dma_start(out=enc_sb[:, 0:2, 0, :],
                                      in_=enc_ap[b, 0][:, 0:2, :])
                    nc.sync.dma_start(out=we_flat[:, 2 * D :],
                                      in_=we_ap[:, 2 * D :])
                    nc.sync.dma_start(out=enc_sb[:, 2:4, 0, :],
                                      in_=enc_ap[b, 0][:, 2:4, :])
                    nc.sync.dma_start(out=c_sb, in_=c_ap)
                    nc.sync.dma_start(out=v_sb, in_=v_ap)
                    rest = range(1, SC)
                else:
                    rest = range(SC)
                for sb in rest:
                    nc.sync.dma_start(
                        out=enc_sb[:, :, sb, :],
                        in_=enc_ap[b, sb],
                    )

                if b == BL - 1:
                    sc_lo = scp.tile([P, ST], f32, name="scores_lo", tag="sc")
                    sc_hi = scp.tile([P, ST], f32, name="scores_hi", tag="sc")
                    scores = [sc_lo, sc_lo, sc_hi, sc_hi]
                else:
                    scores = scp.tile([P, ST], f32, name=f"scores_b{b}",
                                      tag="sc")

                def tanh_en(ps_t, d, s):
                    en = ensp.tile([P, ST], DT, name=f"en_b{b}d{d}s{s}",
                                   tag="ens")
                    nc.scalar.activation(
                        en,
                        ps_t,
                        mybir.ActivationFunctionType.Tanh,
                        bias=c_sb[:, b * DC + d : b * DC + d + 1],
                    )
                    return en

                if b == 0:
                    # s-outer / e-grouped / d-inner: matmuls for s-block s
                    # need only we + enc s-block s
                    en_by_d = [[None] * SC for _ in range(DC)]
                    for s in range(SC):
                        ps4 = [
                            psp.tile([P, ST], f32, name=f"ps_b0s{s}d{d}",
                                     tag="ps")
                            for d in range(DC)
                        ]
                        for e in range(EC):
                            for d in range(DC):
                                nc.tensor.matmul(
                                    ps4[d],
                                    lhsT=we_sb[:, e, d * P : (d + 1) * P],
                                    rhs=enc_sb[:, e, s, :],
                                    start=(e == 0),
                                    stop=(e == EC - 1),
                                )
                                if e == EC - 1:
                                    en_by_d[d][s] = tanh_en(ps4[d], d, s)
                    for d in range(DC):
                        rounds.append((scores, d, en_by_d[d], b))
                else:
                    for d in range(DC):
                        ps4 = [
                            psp.tile([P, ST], f32, name=f"ps_b{b}d{d}s{s}",
                                     tag="ps")
                            for s in range(SC)
                        ]
                        ens = [None] * SC
                        if b == BL - 1 and d == DC - 1:
                            # s-grouped: each s-block stops early; its tanh
                            # overlaps the remaining matmuls (short tail)
                            order = [(e, s) for s in range(SC)
                                     for e in range(EC)]
                        else:
                            # e-outer: 4 consecutive matmuls share lhsT
                            order = [(e, s) for e in range(EC)
                                     for s in range(SC)]
                        for i, (e, s) in enumerate(order):
                            nc.tensor.matmul(
                                ps4[s],
                                lhsT=we_sb[:, e, d * P : (d + 1) * P],
                                rhs=enc_sb[:, e, s, :],
                                start=(e == 0),
                                stop=(e == EC - 1),
                            )
                            if e == EC - 1:
                                ens[s] = tanh_en(ps4[s], d, s)
                            if i == 13 and (d == 0 or b == BL - 1):
                                # one 4-round block per batch (fewer strip<->
                                # full-width turnarounds); the last batch
                                # drains per-d so its tail stays short
                                slot(4)
                        rounds.append((scores, d, ens, b))

            # final round (last batch, d3): all 4 strip matvecs, then row
            # copies alternating DVE/ACT. The lo tile's copies only wait on
            # its own strips, so they overlap the hi strips' matvecs.
            tiles, d, ens, b = rounds.pop(0)
            assert d == DC - 1 and not rounds and isinstance(tiles, list)
            for s in range(SC):
                nc.tensor.matmul(
                    tiles[s][32 * s : 32 * s + 1, :],
                    lhsT=v_sb[:, d : d + 1],
                    rhs=ens[s],
                    start=False,
                    stop=True,
                    tile_position=(0, 32 * s),
                )
            outw = outp.tile([P, ST], f32, name="outw_last", tag="outw")
            for s in range(SC):
                src = tiles[s][32 * s : 32 * s + 1, :]
                dst = outw[32 * s : 32 * s + 1, :]
                if s % 2 == 0:
                    nc.vector.tensor_scalar_mul(dst, src, 1.0)
                else:
                    nc.scalar.copy(dst, src)
            nc.sync.dma_start(
                out=out_ap[b].rearrange("(r s) -> r s", r=SC),
                in_=outw[0 : 32 * (SC - 1) + 1 : 32, :],
            )

    nc.compile()
    return nc


def _get_nc():
    global _COMPILED
    if _COMPILED is None:
        _COMPILED = _build()
    return _COMPILED


def _prep_in_maps(hidden, encoder_outputs, attn_W, attn_b, v_w):
    import ml_dtypes

    hidden = np.asarray(hidden, dtype=np.float32)
    encoder_outputs = np.asarray(encoder_outputs, dtype=np.float32)
    attn_W = np.asarray(attn_W, dtype=np.float32)
    attn_b = np.asarray(attn_b, dtype=np.float32)
    v_w = np.asarray(v_w, dtype=np.float32)

    c_full = hidden @ attn_W[:D] + attn_b            # [B, D]
    # We p-major: we[p, e*D + d] = We[e*P + p, d] -> 4 KB DRAM runs
    we = np.ascontiguousarray(
        attn_W[D:].reshape(EC, P, D).transpose(1, 0, 2).reshape(P, EC * D)
    ).astype(ml_dtypes.bfloat16)
    v = np.ascontiguousarray(v_w.reshape(DC, P).T).astype(ml_dtypes.bfloat16)

    in_maps = []
    for i in range(N_CORES):
        lo = i * BL
        enc_bes = encoder_outputs[:, lo : lo + BL, :].transpose(1, 2, 0)
        # [BL, E, S] -> [BL, SC, P, EC, ST]: partition-major per s-block so
        # each partition's DMA run is EC*ST*2 = 4 KB of contiguous DRAM
        enc_t = np.ascontiguousarray(
            enc_bes.reshape(BL, EC, P, SC, ST).transpose(0, 3, 2, 1, 4)
        ).astype(ml_dtypes.bfloat16)
        c_shard = c_full[lo : lo + BL]               # [BL, D]
        c = np.ascontiguousarray(
            c_shard.reshape(BL, DC, P).transpose(2, 0, 1).reshape(P, BL * DC)
        )                                            # [P, BL*DC]
        in_maps.append({"enc_t": enc_t, "we": we, "c": c, "v": v})
    return in_maps


def run_full(inputs: dict, trace: bool = False):
    """Run on 8 cores; returns (full_output [B,S], BassKernelResults)."""
    nc = _get_nc()
    in_maps = _prep_in_maps(**inputs)
    res = run_bass_kernel_spmd(
        nc, in_maps, list(range(N_CORES)), trace=trace
    )
    scores = np.concatenate(
        [res.results[i]["out"] for i in range(N_CORES)], axis=0
    )                                                # [B, S] raw scores
    scores = scores - scores.max(axis=1, keepdims=True)
    ex = np.exp(scores)
    out = (ex / ex.sum(axis=1, keepdims=True)).astype(np.float32)
    return out, res


def kernel(**inputs) -> np.ndarray:
    out, _ = run_full(inputs)
    return out


# revision 28
# speedup vs baseline: 1.1920x; 1.0002x over previous
"""Bahdanau-style attention scores kernel for 8 TRN2 NeuronCores.

Reference math (B=64, S=2048, E=512, D=512):
    Wh = attn_W[:D]; We = attn_W[D:]
    h_proj = hidden @ Wh                                  # [B, D]
    e_proj[b,s,:] = enc[b,s,:] @ We                       # [B, S, D]
    energy = tanh(h_proj[:,None,:] + e_proj + attn_b)     # [B, S, D]
    scores = energy @ v_w                                 # [B, S]
    out = softmax(scores, axis=1)

Sharding: data-parallel over batch, 8 batches per core.
Host precomputes c = hidden @ Wh + attn_b (tiny: 33 MFLOP), converts
enc/We to bf16 (halves HBM traffic; absmax_rel ~5e-3, gate is 2e-2),
lays enc out partition-major per s-block ([SC, P, EC, ST], giving the
DMA 4 KB contiguous DRAM runs instead of 1 KB -> full ~358 GB/s at
startup) and We as one p-major [P, EC*D] block, and applies the final
softmax to the raw scores the device returns (scores are 0.2% of the
FLOPs; this removes the psum zero opener/closer matmuls and the
exp/reduce/reciprocal tail chain from the device critical path).

Per-core kernel (b = 0..7 local batches):
  b0 runs s-outer / e-grouped / d-inner so matmuls only need the
  s-blocks that have already landed; warm-up matmuls (dummy data)
  bridge the ~5 us DMA latency+transfer window for we+s0 and keep the
  PE continuously busy so the HAM clock-gate releases (1.2 -> 2.4 GHz)
  by the time real work starts. b1..b7 run d-outer / e-outer (4
  consecutive matmuls share lhsT).
  PSUM: 6 single-bank [128, 512] tiles (+2 score banks); each (d, s)
  accumulation gets its own bank and tanh ACT call ([128,512] f32 ->
  bf16 energy, bias c[b,d]) for smooth bank rotation.
  Score matvecs (v_d.T @ energyT, 4 s-strips concurrent via
  tile_position column groups) are deferred one batch and flushed as
  one 4-round block at MM #14 of the next batch's d0, where rounds
  pipeline at the ~215 ns full-width issue rate (one block per batch
  minimizes strip<->full-width array turnarounds). After a batch's
  last round, DVE copies the 4 score rows psum->sbuf and a small DMA
  writes them out; the last batch instead drains per-d so its tail
  stays short.
  The last batch's scores live in two psum banks (strips s0,s1 vs
  s2,s3) so the tail's row copies don't serialize against later strip
  matvecs (Tile's WAR tracking is tile-granular), its last d-block is
  s-grouped so tanh overlaps the matmuls, and the final copies
  alternate DVE/ACT: the exposed tail is ~2 us.
"""

import numpy as np

import concourse.bass as bass  # noqa: F401  (engine namespaces via nc)
import concourse.mybir as mybir
import concourse.tile as tile
from concourse import bacc
from concourse.bass_utils import run_bass_kernel_spmd

N_CORES = 8
B, S, E, D = 64, 2048, 512, 512
BL = B // N_CORES          # local batches per core
P = 128                    # partition tile
EC = E // P                # e chunks (4)
DC = D // P                # d chunks (4)
ST = 512                   # s tile (free dim per matmul; one PSUM bank f32)
SC = S // ST               # s tiles (4)

DT = mybir.dt.bfloat16     # matmul input dtype (enc, We, v, energy)

_COMPILED = None  # nc cache within the process


def _build(warmup=11, enc_bufs=4, psp_bufs=6, ens_bufs=24):
    nc = bacc.Bacc(
        "TRN2", target_bir_lowering=False, debug=False, num_devices=N_CORES
    )
    f32 = mybir.dt.float32

    enc_ap = nc.dram_tensor(
        "enc_t", [BL, SC, P, EC, ST], DT, kind="ExternalInput"
    ).ap()
    we_ap = nc.dram_tensor("we", [P, EC * D], DT, kind="ExternalInput").ap()
    c_ap = nc.dram_tensor("c", [P, BL * DC], f32, kind="ExternalInput").ap()
    v_ap = nc.dram_tensor("v", [P, DC], DT, kind="ExternalInput").ap()
    out_ap = nc.dram_tensor("out", [BL, S], f32, kind="ExternalOutput").ap()

    with tile.TileContext(nc) as tc:
        with (
            tc.tile_pool(name="singles", bufs=1) as singles,
            tc.tile_pool(name="encp", bufs=enc_bufs) as encp,
            tc.tile_pool(name="ensp", bufs=ens_bufs) as ensp,
            tc.tile_pool(name="outp", bufs=2) as outp,
            tc.tile_pool(name="warmp", bufs=1) as warmp,
            tc.tile_pool(name="psp", bufs=psp_bufs, space="PSUM") as psp,
            tc.tile_pool(name="scp", bufs=2, space="PSUM") as scp,
        ):
            # ---- PE warm-up: keep the PE busy while we + the first enc
            # s-block land (dummy data, never read). gpsimd does the memset
            # (its preamble retires earliest).
            wsb = warmp.tile([P, ST], DT)
            nc.gpsimd.memset(wsb, 0.0)
            wps = psp.tile([P, ST], f32, name="warm_ps", tag="ps")
            for _ in range(warmup):
                nc.tensor.matmul(wps, lhsT=wsb[:, 0:P], rhs=wsb,
                                 start=True, stop=True)

            we_sb = singles.tile([P, EC, D], DT)
            c_sb = singles.tile([P, BL * DC], f32)
            v_sb = singles.tile([P, DC], DT)

            rounds = []  # FIFO of (tiles-or-tile, d, [en(d,s) for s], batch)

            def emit_out(scores_t, b):
                # stage the 4 score rows to SBUF (DVE) and DMA them out;
                # host does the softmax.
                outw = outp.tile([P, ST], f32, name=f"outw_b{b}", tag="outw")
                for s in range(SC):
                    nc.vector.tensor_scalar_mul(
                        outw[32 * s : 32 * s + 1, :],
                        scores_t[32 * s : 32 * s + 1, :],
                        1.0,
                    )
                nc.sync.dma_start(
                    out=out_ap[b].rearrange("(r s) -> r s", r=SC),
                    in_=outw[0 : 32 * (SC - 1) + 1 : 32, :],
                )

            def flush_one():
                scores_t, d, ens, b = rounds.pop(0)
                tiles = scores_t if isinstance(scores_t, list) else (
                    [scores_t] * SC)
                for s in range(SC):
                    nc.tensor.matmul(
                        tiles[s][32 * s : 32 * s + 1, :],
                        lhsT=v_sb[:, d : d + 1],
                        rhs=ens[s],
                        start=(d == 0),
                        stop=(d == DC - 1),
                        tile_position=(0, 32 * s),
                    )
                if d == DC - 1 and not isinstance(scores_t, list):
                    emit_out(scores_t, b)

            def slot(k=2):
                for _ in range(min(k, len(rounds))):
                    flush_one()

            for b in range(BL):
                enc_sb = encp.tile([P, EC, SC, ST], DT, name=f"enc_b{b}",
                                   tag="enc")
                if b == 0:
                    # split we and s0 so the first matmuls' deps land early
                    we_flat = we_sb.rearrange("p e d -> p (e d)")
                    nc.sync.dma_start(out=we_flat[:, 0 : 2 * D],
                                      in_=we_ap[:, 0 : 2 * D])
                    nc.sync.dma_start(out=enc_sb[:, 0:2, 0, :],
                                      in_=enc_ap[b, 0][:, 0:2, :])
                    nc.sync.dma_start(out=we_flat[:, 2 * D :],
                                      in_=we_ap[:, 2 * D :])
                    nc.sync.dma_start(out=enc_sb[:, 2:4, 0, :],
                                      in_=enc_ap[b, 0][:, 2:4, :])
                    # s1 before c/v: b0 needs it right after s0 (c's first
                    # use is the first tanh, v's the first matvec flush)
                    nc.sync.dma_start(out=enc_sb[:, :, 1, :],
                                      in_=enc_ap[b, 1])
                    nc.sync.dma_start(out=c_sb, in_=c_ap)
                    nc.sync.dma_start(out=v_sb, in_=v_ap)
                    rest = range(2, SC)
                else:
                    rest = range(SC)
                for sb in rest:
                    nc.sync.dma_start(
                        out=enc_sb[:, :, sb, :],
                        in_=enc_ap[b, sb],
                    )

                if b == BL - 1:
                    sc_lo = scp.tile([P, ST], f32, name="scores_lo", tag="sc")
                    sc_hi = scp.tile([P, ST], f32, name="scores_hi", tag="sc")
                    scores = [sc_lo, sc_lo, sc_hi, sc_hi]
                else:
                    scores = scp.tile([P, ST], f32, name=f"scores_b{b}",
                                      tag="sc")

                def tanh_en(ps_t, d, s):
                    en = ensp.tile([P, ST], DT, name=f"en_b{b}d{d}s{s}",
                                   tag="ens")
                    nc.scalar.activation(
                        en,
                        ps_t,
                        mybir.ActivationFunctionType.Tanh,
                        bias=c_sb[:, b * DC + d : b * DC + d + 1],
                    )
                    return en

                if b == 0:
                    # s-outer / e-grouped / d-inner: matmuls for s-block s
                    # need only we + enc s-block s
                    en_by_d = [[None] * SC for _ in range(DC)]
                    for s in range(SC):
                        ps4 = [
                            psp.tile([P, ST], f32, name=f"ps_b0s{s}d{d}",
                                     tag="ps")
                            for d in range(DC)
                        ]
                        for e in range(EC):
                            for d in range(DC):
                                nc.tensor.matmul(
                                    ps4[d],
                                    lhsT=we_sb[:, e, d * P : (d + 1) * P],
                                    rhs=enc_sb[:, e, s, :],
                                    start=(e == 0),
                                    stop=(e == EC - 1),
                                )
                                if e == EC - 1:
                                    en_by_d[d][s] = tanh_en(ps4[d], d, s)
                    for d in range(DC):
                        rounds.append((scores, d, en_by_d[d], b))
                else:
                    for d in range(DC):
                        ps4 = [
                            psp.tile([P, ST], f32, name=f"ps_b{b}d{d}s{s}",
                                     tag="ps")
                            for s in range(SC)
                        ]
                        ens = [None] * SC
                        if b == BL - 1 and d == DC - 1:
                            # s-grouped: each s-block stops early; its tanh
                            # overlaps the remaining matmuls (short tail)
                            order = [(e, s) for s in range(SC)
                                     for e in range(EC)]
                        else:
                            # e-outer: 4 consecutive matmuls share lhsT
                            order = [(e, s) for e in range(EC)
                                     for s in range(SC)]
                        for i, (e, s) in enumerate(order):
                            nc.tensor.matmul(
                                ps4[s],
                                lhsT=we_sb[:, e, d * P : (d + 1) * P],
                                rhs=enc_sb[:, e, s, :],
                                start=(e == 0),
                                stop=(e == EC - 1),
                            )
                            if e == EC - 1:
                                ens[s] = tanh_en(ps4[s], d, s)
                            if i == 13 and (d == 0 or b == BL - 1):
                                # one 4-round block per batch (fewer strip<->
                                # full-width turnarounds); the last batch
                                # drains per-d so its tail stays short
                                slot(4)
                        rounds.append((scores, d, ens, b))

            # final round (last batch, d3): all 4 strip matvecs, then row
            # copies alternating DVE/ACT. The lo tile's copies only wait on
            # its own strips, so they overlap the hi strips' matvecs.
            tiles, d, ens, b = rounds.pop(0)
            assert d == DC - 1 and not rounds and isinstance(tiles, list)
            for s in range(SC):
                nc.tensor.matmul(
                    tiles[s][32 * s : 32 * s + 1, :],
                    lhsT=v_sb[:, d : d + 1],
                    rhs=ens[s],
                    start=False,
                    stop=True,
                    tile_position=(0, 32 * s),
                )
            outw = outp.tile([P, ST], f32, name="outw_last", tag="outw")
            for s in range(SC):
                src = tiles[s][32 * s : 32 * s + 1, :]
                dst = outw[32 * s : 32 * s + 1, :]
                if s % 2 == 0:
                    nc.vector.tensor_scalar_mul(dst, src, 1.0)
                else:
                    nc.scalar.copy(dst, src)
            nc.sync.dma_start(
                out=out_ap[b].rearrange("(r s) -> r s", r=SC),
                in_=outw[0 : 32 * (SC - 1) + 1 : 32, :],
            )

    nc.compile()
    return nc


def _get_nc():
    global _COMPILED
    if _COMPILED is None:
        _COMPILED = _build()
    return _COMPILED


def _prep_in_maps(hidden, encoder_outputs, attn_W, attn_b, v_w):
    import ml_dtypes

    hidden = np.asarray(hidden, dtype=np.float32)
    encoder_outputs = np.asarray(encoder_outputs, dtype=np.float32)
    attn_W = np.asarray(attn_W, dtype=np.float32)
    attn_b = np.asarray(attn_b, dtype=np.float32)
    v_w = np.asarray(v_w, dtype=np.float32)

    c_full = hidden @ attn_W[:D] + attn_b            # [B, D]
    # We p-major: we[p, e*D + d] = We[e*P + p, d] -> 4 KB DRAM runs
    we = np.ascontiguousarray(
        attn_W[D:].reshape(EC, P, D).transpose(1, 0, 2).reshape(P, EC * D)
    ).astype(ml_dtypes.bfloat16)
    v = np.ascontiguousarray(v_w.reshape(DC, P).T).astype(ml_dtypes.bfloat16)

    in_maps = []
    for i in range(N_CORES):
        lo = i * BL
        enc_bes = encoder_outputs[:, lo : lo + BL, :].transpose(1, 2, 0)
        # [BL, E, S] -> [BL, SC, P, EC, ST]: partition-major per s-block so
        # each partition's DMA run is EC*ST*2 = 4 KB of contiguous DRAM
        enc_t = np.ascontiguousarray(
            enc_bes.reshape(BL, EC, P, SC, ST).transpose(0, 3, 2, 1, 4)
        ).astype(ml_dtypes.bfloat16)
        c_shard = c_full[lo : lo + BL]               # [BL, D]
        c = np.ascontiguousarray(
            c_shard.reshape(BL, DC, P).transpose(2, 0, 1).reshape(P, BL * DC)
        )                                            # [P, BL*DC]
        in_maps.append({"enc_t": enc_t, "we": we, "c": c, "v": v})
    return in_maps


def run_full(inputs: dict, trace: bool = False):
    """Run on 8 cores; returns (full_output [B,S], BassKernelResults)."""
    nc = _get_nc()
    in_maps = _prep_in_maps(**inputs)
    res = run_bass_kernel_spmd(
        nc, in_maps, list(range(N_CORES)), trace=trace
    )
    scores = np.concatenate(
        [res.results[i]["out"] for i in range(N_CORES)], axis=0
    )                                                # [B, S] raw scores
    scores = scores - scores.max(axis=1, keepdims=True)
    ex = np.exp(scores)
    out = (ex / ex.sum(axis=1, keepdims=True)).astype(np.float32)
    return out, res


def kernel(**inputs) -> np.ndarray:
    out, _ = run_full(inputs)
    return out
